# revision 6
# baseline (speedup 1.0000x reference)
"""GSA loss kernel for Trainium2 (8 NeuronCores).

Reference computes, for p = pred.reshape(B, -1) and T = token [B, N, N]:
    s_f  = sum_bij p_bi T_bij
    s_fa = sum_bij p_bi T_bij p_bj
    s_b  = sum_bij (1-p_bi) T_bij
    s_bb = sum_bij (1-p_bi) T_bij (1-p_bj)
    loss = 2 - s_fa/s_f - s_bb/s_b

All four scalars derive from two per-batch column vectors:
    v_bj = sum_i p_bi T_bij      (p-weighted column sums)
    c_bj = sum_i T_bij           (plain column sums)
via  s_f = sum v,  s_fa = sum v*p,  total = sum c,  colp = sum c*p,
     s_b = total - s_f,  s_bb = total - s_f - colp + s_fa.

The contraction over i maps directly onto the tensor engine (contraction
along the partition dim): per 128-row block, one matmul with stationary
lhsT = [p_block, ones] ([128, 2]) against the moving token tile
([128, 512]) accumulates [v; c] partials ([2, 512]) in PSUM.

Sharding: 8 cores = 2 batches x 4 row-slices of 1024 rows. Each core
streams its contiguous 16.75MB token slice once (memory-bound), emits a
[2, 4096] partial; the host sums partials and forms the scalar loss in
float64.
"""

import numpy as np

import concourse.bacc as bacc
import concourse.mybir as mybir
import concourse.tile as tile
from concourse.bass_utils import run_bass_kernel_spmd

B = 2
N = 4096          # HW = 64*64
N_CORES = 8
CORES_PER_BATCH = 4
ROWS_PER_CORE = N // CORES_PER_BATCH   # 1024
NBLK = ROWS_PER_CORE // 128            # 8 row blocks per core
JTILE = 512                            # fp32 matmul moving free-dim max
NJT = N // JTILE                       # 8 column tiles

FP32 = mybir.dt.float32

_NC_CACHE = {}


F32R = mybir.dt.float32r


def _build_nc(dtype=F32R, reps=1, bufs=4, dyn_reps=None, nsplit=2):
    """One core's program.

    dtype: float32r runs the PE at 1 cycle/row (vs 4 for fp32); bits are
    identical to fp32 so the DMA/numpy side is unchanged. Measured
    partial-sum error ~1e-5 (vs 1e-7 fp32) — far inside tolerance — and
    the kernel is DMA-bound at this rate.
    nsplit: column-chunks per 128-row block; 2 -> 1MB DMAs, which
    measured faster than one 2MB DMA per block (48.0 vs 55.8 us/rep).
    `reps` statically re-runs the full workload (each rep overwrites
    PSUM via start=True at its first row block); `dyn_reps` wraps the
    body in a hardware For_i loop instead — both only for repeat-slope
    timing.
    """
    nc = bacc.Bacc("TRN2", target_bir_lowering=False, debug=False)
    tok = nc.dram_tensor("tok", [ROWS_PER_CORE, N], dtype, kind="ExternalInput")
    w = nc.dram_tensor("w", [128, 2 * NBLK], dtype, kind="ExternalInput")
    out = nc.dram_tensor("out", [2, N], FP32, kind="ExternalOutput")

    CW = N // nsplit
    JPC = CW // JTILE  # j-tiles per column chunk

    with tile.TileContext(nc) as tc:
        with (
            tc.tile_pool(name="wp", bufs=1) as wp,
            tc.tile_pool(name="tp", bufs=bufs * nsplit) as tp,
            tc.tile_pool(name="op", bufs=1) as op,
            tc.tile_pool(name="ps", bufs=1, space="PSUM") as ps,
        ):
            wt = wp.tile([128, 2 * NBLK], dtype)
            nc.sync.dma_start(wt[:], w[:])

            psums = []
            for j in range(NJT):
                pst = ps.tile([2, JTILE], FP32, tag=f"ps{j}", name=f"ps{j}")
                psums.append(pst)

            def body(rep):
                for ib in range(NBLK):
                    for s in range(nsplit):
                        t = tp.tile([128, CW], dtype, tag="t", name=f"t{rep}_{ib}_{s}")
                        nc.sync.dma_start(
                            t[:], tok[ib * 128:(ib + 1) * 128, s * CW:(s + 1) * CW]
                        )
                        for jj in range(JPC):
                            jt = s * JPC + jj
                            nc.tensor.matmul(
                                psums[jt][:],
                                wt[:, 2 * ib:2 * ib + 2],
                                t[:, jj * JTILE:(jj + 1) * JTILE],
                                start=(ib == 0),
                                stop=(ib == NBLK - 1),
                            )

            if dyn_reps is not None:
                with tc.For_i(0, dyn_reps, 1):
                    body("d")
            else:
                for rep in range(reps):
                    body(rep)

            ot = op.tile([2, N], FP32)
            for jt in range(NJT):
                nc.vector.tensor_copy(ot[:, jt * JTILE:(jt + 1) * JTILE], psums[jt][:])
            nc.sync.dma_start(out[:], ot[:])

    nc.compile()
    return nc


def get_nc():
    if "nc" not in _NC_CACHE:
        _NC_CACHE["nc"] = _build_nc()
    return _NC_CACHE["nc"]


def make_in_maps(pred, token):
    p = np.asarray(pred, dtype=np.float32).reshape(B, -1)
    token = np.asarray(token, dtype=np.float32)
    in_maps = []
    for c in range(N_CORES):
        b = c // CORES_PER_BATCH
        r0 = (c % CORES_PER_BATCH) * ROWS_PER_CORE
        w = np.empty((128, 2 * NBLK), dtype=np.float32)
        for ib in range(NBLK):
            w[:, 2 * ib] = p[b, r0 + ib * 128: r0 + (ib + 1) * 128]
            w[:, 2 * ib + 1] = 1.0
        in_maps.append({
            "tok": np.ascontiguousarray(token[b, r0:r0 + ROWS_PER_CORE, :]),
            "w": w,
        })
    return in_maps


def finish(pred, results):
    """Host reduction: sum per-core [2, N] partials, form the scalar loss."""
    p = np.asarray(pred, dtype=np.float32).reshape(B, -1).astype(np.float64)
    v = np.zeros((B, N), dtype=np.float64)
    cs = np.zeros((B, N), dtype=np.float64)
    for c in range(N_CORES):
        b = c // CORES_PER_BATCH
        part = results[c]["out"].astype(np.float64)
        v[b] += part[0]
        cs[b] += part[1]
    s_f = v.sum()
    s_fa = (v * p).sum()
    total = cs.sum()
    colp = (cs * p).sum()
    s_b = total - s_f
    s_bb = total - s_f - colp + s_fa
    loss = 2.0 - s_fa / s_f - s_bb / s_b
    return np.array(loss, dtype=np.float32)


def kernel(pred, token):
    nc = get_nc()
    in_maps = make_in_maps(pred, token)
    res = run_bass_kernel_spmd(nc, in_maps, core_ids=list(range(N_CORES))).results
    return finish(pred, res)
